# revision 5
# baseline (speedup 1.0000x reference)
"""BottomRightPool (2D cummax) Trainium2 Bass kernel.

pool[b,c,i,j] = max(x[b,c,:i+1,:j+1])  ==  cummax over H, then over W.

Strategy (per core, data-parallel over the 4096 (b,c) slices):
  - 512 slices/core, processed in 4 chunks of 128 slices.
  - Each chunk is an SBUF tile [128 partitions = slices, 16384 free = (h, w)].
  - cummax along W: one tensor_tensor_scan (op0=op1=max) per row, in place.
  - cummax along H: one strided tensor_tensor_scan per column, in place.
  Both scans run along the free dimension, so no transposes / PSUM / PE work
  is needed, and the DMAs are fully contiguous 64 KiB-per-partition bursts.
"""

import numpy as np

N_CORES = 8
B, C, H, W = 16, 256, 128, 128
S = B * C                    # 4096 independent (b,c) slices
SPC = S // N_CORES           # 512 slices per core
CHUNK = 128                  # slices per SBUF tile (partition dim)
FREE = H * W
NEG = -3.0e38

# Engine assignment per chunk index (cycled): "v" = DVE, "g" = GpSimd.
CHUNK_ENGINES = ("v", "v", "v", "v")


def _build_nc(repeat=None, variant="base", engines=None):
    """Build the per-core Bass program. repeat=None emits the plain kernel;
    repeat=R wraps the whole workload in a hardware For_i loop (benchmarking
    only — output is just rewritten R times).

    variant: "base" | "dma_only" | "scan2x" (scan2x repeats every scan twice —
    idempotent for max, doubles vector-engine work for A/B timing)."""
    import concourse.mybir as mybir
    import concourse.tile as tile
    from concourse import bacc

    engines = engines or CHUNK_ENGINES
    nc = bacc.Bacc(None, target_bir_lowering=False)
    xd = nc.dram_tensor("x", [SPC, H, W], mybir.dt.float32, kind="ExternalInput")
    od = nc.dram_tensor("out", [SPC, H, W], mybir.dt.float32, kind="ExternalOutput")
    MAX = mybir.AluOpType.max
    nscan = 2 if variant == "scan2x" else (0 if variant == "dma_only" else 1)

    with tile.TileContext(nc) as tc:
        with tc.tile_pool(name="io", bufs=2) as pool:

            def body():
                for ci in range(SPC // CHUNK):
                    s0 = ci * CHUNK
                    eng = {"v": nc.vector, "g": nc.gpsimd}[
                        engines[ci % len(engines)]
                    ]
                    t = pool.tile([CHUNK, FREE], mybir.dt.float32)
                    tap = t[:]
                    nc.sync.dma_start(
                        out=tap, in_=xd[s0 : s0 + CHUNK].rearrange("s h w -> s (h w)")
                    )
                    for _ in range(nscan):
                        for r in range(H):
                            seg = t[:, r * W : (r + 1) * W]
                            eng.tensor_tensor_scan(
                                out=seg, data0=seg, data1=seg,
                                initial=NEG, op0=MAX, op1=MAX,
                            )
                    twh = tap.rearrange("p (h w) -> p w h", h=H)
                    for _ in range(nscan):
                        for w in range(W):
                            col = twh[:, w]
                            eng.tensor_tensor_scan(
                                out=col, data0=col, data1=col,
                                initial=NEG, op0=MAX, op1=MAX,
                            )
                    nc.scalar.dma_start(
                        out=od[s0 : s0 + CHUNK].rearrange("s h w -> s (h w)"),
                        in_=tap,
                    )

            if repeat is None:
                body()
            else:
                with tc.For_i(0, repeat, 1):
                    body()
    nc.compile()
    return nc


def make_runner(nc):
    """Compile once; return run(in_maps) -> list of per-core output dicts.

    Mirrors concourse.bass2jax.run_bass_via_pjrt's multi-core path but keeps
    the jitted executable so repeated calls don't re-trace/re-compile.
    Also returns a timed_call(dev_args) for benchmarking.
    """
    import jax
    import concourse.mybir as mybir
    from jax.sharding import Mesh, PartitionSpec
    from jax.experimental.shard_map import shard_map
    from concourse.bass2jax import (
        _bass_exec_p,
        install_neuronx_cc_hook,
        partition_id_tensor,
    )

    install_neuronx_cc_hook()
    assert nc.dbg_addr is None
    partition_name = nc.partition_id_tensor.name if nc.partition_id_tensor else None

    in_names, out_names, out_avals, zero_outs = [], [], [], []
    for alloc in nc.m.functions[0].allocations:
        if not isinstance(alloc, mybir.MemoryLocationSet):
            continue
        name = alloc.memorylocations[0].name
        if alloc.kind == "ExternalInput":
            if name == partition_name:
                continue
            in_names.append(name)
        elif alloc.kind == "ExternalOutput":
            out_names.append(name)
            shape = tuple(alloc.tensor_shape)
            dtype = mybir.dt.np(alloc.dtype)
            out_avals.append(jax.core.ShapedArray(shape, dtype))
            zero_outs.append(np.zeros(shape, dtype))
    n_params = len(in_names)
    n_outs = len(out_avals)
    all_in_names = in_names + out_names
    if partition_name is not None:
        all_in_names = all_in_names + [partition_name]
    donate = tuple(range(n_params, n_params + n_outs))

    def _body(*args):
        operands = list(args)
        if partition_name is not None:
            operands.append(partition_id_tensor())
        outs = _bass_exec_p.bind(
            *operands,
            out_avals=tuple(out_avals),
            in_names=tuple(all_in_names),
            out_names=tuple(out_names),
            lowering_input_output_aliases=(),
            sim_require_finite=True,
            sim_require_nnan=True,
            nc=nc,
        )
        return tuple(outs)

    devices = jax.devices()[:N_CORES]
    mesh = Mesh(np.asarray(devices), ("core",))
    sharded = jax.jit(
        shard_map(
            _body,
            mesh=mesh,
            in_specs=(PartitionSpec("core"),) * (n_params + n_outs),
            out_specs=(PartitionSpec("core"),) * n_outs,
            check_rep=False,
        ),
        donate_argnums=donate,
        keep_unused=True,
    )

    def make_args(in_maps):
        concat_in = [
            np.concatenate([np.asarray(m[name]) for m in in_maps], axis=0)
            for name in in_names
        ]
        concat_zeros = [
            np.zeros((N_CORES * z.shape[0], *z.shape[1:]), z.dtype)
            for z in zero_outs
        ]
        return concat_in + concat_zeros

    def run(in_maps):
        out_arrs = sharded(*make_args(in_maps))
        return [
            {
                name: np.asarray(out_arrs[i]).reshape(
                    N_CORES, *out_avals[i].shape
                )[c]
                for i, name in enumerate(out_names)
            }
            for c in range(N_CORES)
        ]

    return run, sharded, make_args


def _run(x: np.ndarray, trace: bool = False):
    """Returns (full_output, exec_time_ns_or_None)."""
    nc = _build_nc()
    run, _, _ = make_runner(nc)
    xf = np.ascontiguousarray(x, dtype=np.float32).reshape(S, H, W)
    in_maps = [{"x": xf[k * SPC : (k + 1) * SPC]} for k in range(N_CORES)]
    results = run(in_maps)
    out = np.concatenate([r["out"] for r in results], axis=0)
    return out.reshape(B, C, H, W), None


def kernel(x: np.ndarray) -> np.ndarray:
    return _run(x)[0]


# revision 7
# speedup vs baseline: 3.7908x; 3.7908x over previous
"""BottomRightPool (2D cummax) Trainium2 Bass kernel.

pool[b,c,i,j] = max(x[b,c,:i+1,:j+1])  ==  cummax over H, then over W.

Strategy (per core, data-parallel over the 4096 (b,c) slices):
  - 512 slices/core, processed in 4 chunks of 128 slices.
  - Each chunk is an SBUF tile [128 partitions = slices, 16384 free = (h, w)].
  - cummax along W: one tensor_tensor_scan (op0=op1=max) per row, in place.
  - cummax along H: one strided tensor_tensor_scan per column, in place.
  Both scans run along the free dimension, so no transposes / PSUM / PE work
  is needed, and the DMAs are fully contiguous 64 KiB-per-partition bursts.
"""

import numpy as np

N_CORES = 8
B, C, H, W = 16, 256, 128, 128
S = B * C                    # 4096 independent (b,c) slices
SPC = S // N_CORES           # 512 slices per core
CHUNK = 128                  # slices per SBUF tile (partition dim)
FREE = H * W
NEG = -3.0e38

# Engine assignment per chunk index (cycled): "v" = DVE, "g" = GpSimd.
CHUNK_ENGINES = ("v", "v", "v", "v")


def _build_nc(repeat=None, variant="base", engines=None):
    """Build the per-core Bass program. repeat=None emits the plain kernel;
    repeat=R wraps the whole workload in a hardware For_i loop (benchmarking
    only — output is just rewritten R times).

    variant: "base" | "dma_only" | "scan2x" (scan2x repeats every scan twice —
    idempotent for max, doubles vector-engine work for A/B timing)."""
    import concourse.mybir as mybir
    import concourse.tile as tile
    from concourse import bacc

    engines = engines or CHUNK_ENGINES
    nc = bacc.Bacc(None, target_bir_lowering=False)
    xd = nc.dram_tensor("x", [SPC, H, W], mybir.dt.float32, kind="ExternalInput")
    od = nc.dram_tensor("out", [SPC, H, W], mybir.dt.float32, kind="ExternalOutput")
    MAX = mybir.AluOpType.max
    nscan = 2 if variant == "scan2x" else (0 if variant == "dma_only" else 1)
    nscan_w = 0 if variant == "honly" else nscan
    nscan_h = 0 if variant == "wonly" else nscan

    with tile.TileContext(nc) as tc:
        with tc.tile_pool(name="io", bufs=2) as pool:

            def body():
                for ci in range(SPC // CHUNK):
                    s0 = ci * CHUNK
                    eng = {"v": nc.vector, "g": nc.gpsimd}[
                        engines[ci % len(engines)]
                    ]
                    t = pool.tile([CHUNK, FREE], mybir.dt.float32)
                    tap = t[:]
                    nc.sync.dma_start(
                        out=tap, in_=xd[s0 : s0 + CHUNK].rearrange("s h w -> s (h w)")
                    )
                    for _ in range(nscan_w):
                        for r in range(H):
                            seg = t[:, r * W : (r + 1) * W]
                            eng.tensor_tensor_scan(
                                out=seg, data0=seg, data1=seg,
                                initial=NEG, op0=MAX, op1=MAX,
                            )
                    twh = tap.rearrange("p (h w) -> p w h", h=H)
                    for _ in range(nscan_h):
                        for w in range(W):
                            col = twh[:, w]
                            eng.tensor_tensor_scan(
                                out=col, data0=col, data1=col,
                                initial=NEG, op0=MAX, op1=MAX,
                            )
                    nc.scalar.dma_start(
                        out=od[s0 : s0 + CHUNK].rearrange("s h w -> s (h w)"),
                        in_=tap,
                    )

            if repeat is None:
                body()
            else:
                with tc.For_i(0, repeat, 1):
                    body()
    nc.compile()
    return nc


def make_runner(nc):
    """Compile once; return run(in_maps) -> list of per-core output dicts.

    Mirrors concourse.bass2jax.run_bass_via_pjrt's multi-core path but keeps
    the jitted executable so repeated calls don't re-trace/re-compile.
    Also returns a timed_call(dev_args) for benchmarking.
    """
    import jax
    import concourse.mybir as mybir
    from jax.sharding import Mesh, PartitionSpec
    from jax.experimental.shard_map import shard_map
    from concourse.bass2jax import (
        _bass_exec_p,
        install_neuronx_cc_hook,
        partition_id_tensor,
    )

    install_neuronx_cc_hook()
    assert nc.dbg_addr is None
    partition_name = nc.partition_id_tensor.name if nc.partition_id_tensor else None

    in_names, out_names, out_avals, zero_outs = [], [], [], []
    for alloc in nc.m.functions[0].allocations:
        if not isinstance(alloc, mybir.MemoryLocationSet):
            continue
        name = alloc.memorylocations[0].name
        if alloc.kind == "ExternalInput":
            if name == partition_name:
                continue
            in_names.append(name)
        elif alloc.kind == "ExternalOutput":
            out_names.append(name)
            shape = tuple(alloc.tensor_shape)
            dtype = mybir.dt.np(alloc.dtype)
            out_avals.append(jax.core.ShapedArray(shape, dtype))
            zero_outs.append(np.zeros(shape, dtype))
    n_params = len(in_names)
    n_outs = len(out_avals)
    all_in_names = in_names + out_names
    if partition_name is not None:
        all_in_names = all_in_names + [partition_name]
    donate = tuple(range(n_params, n_params + n_outs))

    def _body(*args):
        operands = list(args)
        if partition_name is not None:
            operands.append(partition_id_tensor())
        outs = _bass_exec_p.bind(
            *operands,
            out_avals=tuple(out_avals),
            in_names=tuple(all_in_names),
            out_names=tuple(out_names),
            lowering_input_output_aliases=(),
            sim_require_finite=True,
            sim_require_nnan=True,
            nc=nc,
        )
        return tuple(outs)

    devices = jax.devices()[:N_CORES]
    mesh = Mesh(np.asarray(devices), ("core",))
    sharded = jax.jit(
        shard_map(
            _body,
            mesh=mesh,
            in_specs=(PartitionSpec("core"),) * (n_params + n_outs),
            out_specs=(PartitionSpec("core"),) * n_outs,
            check_rep=False,
        ),
        donate_argnums=donate,
        keep_unused=True,
    )

    def make_args(in_maps):
        concat_in = [
            np.concatenate([np.asarray(m[name]) for m in in_maps], axis=0)
            for name in in_names
        ]
        concat_zeros = [
            np.zeros((N_CORES * z.shape[0], *z.shape[1:]), z.dtype)
            for z in zero_outs
        ]
        return concat_in + concat_zeros

    def run(in_maps):
        out_arrs = sharded(*make_args(in_maps))
        return [
            {
                name: np.asarray(out_arrs[i]).reshape(
                    N_CORES, *out_avals[i].shape
                )[c]
                for i, name in enumerate(out_names)
            }
            for c in range(N_CORES)
        ]

    return run, sharded, make_args


def _run(x: np.ndarray, trace: bool = False):
    """Returns (full_output, exec_time_ns_or_None)."""
    nc = _build_nc()
    run, _, _ = make_runner(nc)
    xf = np.ascontiguousarray(x, dtype=np.float32).reshape(S, H, W)
    in_maps = [{"x": xf[k * SPC : (k + 1) * SPC]} for k in range(N_CORES)]
    results = run(in_maps)
    out = np.concatenate([r["out"] for r in results], axis=0)
    return out.reshape(B, C, H, W), None


def kernel(x: np.ndarray) -> np.ndarray:
    return _run(x)[0]


# revision 8
# speedup vs baseline: 4.8127x; 1.2696x over previous
"""BottomRightPool (2D cummax) Trainium2 Bass kernel.

pool[b,c,i,j] = max(x[b,c,:i+1,:j+1])  ==  cummax over H, then over W.

Key identity: pool rows are non-decreasing along w, so
    pool[i, :] = scan_j ( state = max(state, x[i, j], pool[i-1, j]) )
because cummax_w(pool[i-1, :]) == pool[i-1, :].  tensor_tensor_scan computes
exactly  state = max(max(data0, state), data1), so ONE scan instruction per
row (data0 = x row i, data1 = pool row i-1) performs BOTH cummax passes.

Layout (per core, data-parallel over the 4096 (b,c) slices):
  - 512 slices/core; tiles of [128 partitions = slices, HB*128 free = (h, w)].
  - 4 slice-chunks x (128/HB) h-blocks; the row recurrence chains across
    h-blocks via data1 = previous block's last output row.
  - All scans out-of-place (in-place scans measured ~2.2x slower).
  - DMAs are contiguous HB*512B-per-partition bursts (4 MiB per transfer).
"""

import numpy as np

N_CORES = 8
B, C, H, W = 16, 256, 128, 128
S = B * C                    # 4096 independent (b,c) slices
SPC = S // N_CORES           # 512 slices per core
CHUNK = 128                  # slices per tile (partition dim)
HB = 64                      # rows per h-block tile
NEG = -3.0e38


def _build_nc(repeat=None):
    """Build the per-core Bass program. repeat=None emits the plain kernel;
    repeat=R wraps the whole workload in a hardware For_i loop (benchmarking
    only — output is just rewritten R times)."""
    import concourse.mybir as mybir
    import concourse.tile as tile
    from concourse import bacc

    nc = bacc.Bacc(None, target_bir_lowering=False)
    xd = nc.dram_tensor("x", [SPC, H, W], mybir.dt.float32, kind="ExternalInput")
    od = nc.dram_tensor("out", [SPC, H, W], mybir.dt.float32, kind="ExternalOutput")
    MAX = mybir.AluOpType.max

    with tile.TileContext(nc) as tc:
        with tc.tile_pool(name="ina", bufs=2) as pa, tc.tile_pool(
            name="outb", bufs=3
        ) as pb:

            def body():
                for ci in range(SPC // CHUNK):
                    s0 = ci * CHUNK
                    prev_last = None  # [128, 1, 128] AP: pool row above this block
                    for hb in range(H // HB):
                        h0 = hb * HB
                        A = pa.tile([CHUNK, HB * W], mybir.dt.float32)
                        Bt = pb.tile([CHUNK, HB * W], mybir.dt.float32)
                        nc.sync.dma_start(
                            out=A[:],
                            in_=xd[s0 : s0 + CHUNK, h0 : h0 + HB].rearrange(
                                "s h w -> s (h w)"
                            ),
                        )
                        for r in range(HB):
                            row = slice(r * W, (r + 1) * W)
                            if r == 0 and prev_last is None:
                                data1 = A[:, row]
                            elif r == 0:
                                data1 = prev_last
                            else:
                                data1 = Bt[:, (r - 1) * W : r * W]
                            nc.vector.tensor_tensor_scan(
                                out=Bt[:, row],
                                data0=A[:, row],
                                data1=data1,
                                initial=NEG,
                                op0=MAX,
                                op1=MAX,
                            )
                        prev_last = Bt[:, (HB - 1) * W : HB * W]
                        nc.scalar.dma_start(
                            out=od[s0 : s0 + CHUNK, h0 : h0 + HB].rearrange(
                                "s h w -> s (h w)"
                            ),
                            in_=Bt[:],
                        )

            if repeat is None:
                body()
            else:
                with tc.For_i(0, repeat, 1):
                    body()
    nc.compile()
    return nc


def make_runner(nc):
    """Compile once; return run(in_maps) plus the raw jitted callable.

    Mirrors concourse.bass2jax.run_bass_via_pjrt's multi-core path but keeps
    the jitted executable so repeated calls don't re-trace/re-compile.
    """
    import jax
    import concourse.mybir as mybir
    from jax.sharding import Mesh, PartitionSpec
    from jax.experimental.shard_map import shard_map
    from concourse.bass2jax import (
        _bass_exec_p,
        install_neuronx_cc_hook,
        partition_id_tensor,
    )

    install_neuronx_cc_hook()
    assert nc.dbg_addr is None
    partition_name = nc.partition_id_tensor.name if nc.partition_id_tensor else None

    in_names, out_names, out_avals, zero_outs = [], [], [], []
    for alloc in nc.m.functions[0].allocations:
        if not isinstance(alloc, mybir.MemoryLocationSet):
            continue
        name = alloc.memorylocations[0].name
        if alloc.kind == "ExternalInput":
            if name == partition_name:
                continue
            in_names.append(name)
        elif alloc.kind == "ExternalOutput":
            out_names.append(name)
            shape = tuple(alloc.tensor_shape)
            dtype = mybir.dt.np(alloc.dtype)
            out_avals.append(jax.core.ShapedArray(shape, dtype))
            zero_outs.append(np.zeros(shape, dtype))
    n_params = len(in_names)
    n_outs = len(out_avals)
    all_in_names = in_names + out_names
    if partition_name is not None:
        all_in_names = all_in_names + [partition_name]
    donate = tuple(range(n_params, n_params + n_outs))

    def _body(*args):
        operands = list(args)
        if partition_name is not None:
            operands.append(partition_id_tensor())
        outs = _bass_exec_p.bind(
            *operands,
            out_avals=tuple(out_avals),
            in_names=tuple(all_in_names),
            out_names=tuple(out_names),
            lowering_input_output_aliases=(),
            sim_require_finite=True,
            sim_require_nnan=True,
            nc=nc,
        )
        return tuple(outs)

    devices = jax.devices()[:N_CORES]
    mesh = Mesh(np.asarray(devices), ("core",))
    sharded = jax.jit(
        shard_map(
            _body,
            mesh=mesh,
            in_specs=(PartitionSpec("core"),) * (n_params + n_outs),
            out_specs=(PartitionSpec("core"),) * n_outs,
            check_rep=False,
        ),
        donate_argnums=donate,
        keep_unused=True,
    )

    def make_args(in_maps):
        concat_in = [
            np.concatenate([np.asarray(m[name]) for m in in_maps], axis=0)
            for name in in_names
        ]
        concat_zeros = [
            np.zeros((N_CORES * z.shape[0], *z.shape[1:]), z.dtype)
            for z in zero_outs
        ]
        return concat_in + concat_zeros

    def run(in_maps):
        out_arrs = sharded(*make_args(in_maps))
        return [
            {
                name: np.asarray(out_arrs[i]).reshape(
                    N_CORES, *out_avals[i].shape
                )[c]
                for i, name in enumerate(out_names)
            }
            for c in range(N_CORES)
        ]

    return run, sharded, make_args


def _run(x: np.ndarray, trace: bool = False):
    """Returns (full_output, exec_time_ns_or_None)."""
    nc = _build_nc()
    run, _, _ = make_runner(nc)
    xf = np.ascontiguousarray(x, dtype=np.float32).reshape(S, H, W)
    in_maps = [{"x": xf[k * SPC : (k + 1) * SPC]} for k in range(N_CORES)]
    results = run(in_maps)
    out = np.concatenate([r["out"] for r in results], axis=0)
    return out.reshape(B, C, H, W), None


def kernel(x: np.ndarray) -> np.ndarray:
    return _run(x)[0]
